# revision 1
# baseline (speedup 1.0000x reference)
"""Trainium2 Bass kernel for nn_DiversificationBlock.

Semantics (per (b, c) image of shape 56x56):
  peak  = max over the image
  pc    = (act == peak)                       # peak indicator
  full  = 3x3-broadcast of an 18x18 patch mask, zero-padded to 56x56
  maskb = pc ? rc : full                      # 0/1 mask
  out   = maskb ? act * 0.1 : act

Sharding: pure data parallel over the batch dim (32 -> 4 per core,
8 cores).  Per core we flatten (b=4, c=256) -> 1024 images, put 128
images on the 128 SBUF partitions per tile (8 tiles), and keep the
56*56=3136 pixels on the free dimension.

Walrus codegen allows exactly ONE sync-wait per instruction (Tile's
dead-wait eliminator is disabled, and Tile emits a semaphore wait for
every cross-instruction dep, including same-engine slot WAW).  Every
engine therefore runs tiny "joiner" reads that advance its vector clock
over foreign semaphores so each real op needs at most one wait.
Pipelining: loads go through the gpsimd SWDGE queue, stores through the
scalar engine's HWDGE ring, and the compute-gated Pool observers of
iteration t are deferred past iteration t+1's loads so the DMA queue
never stalls on compute.
"""

import numpy as np
from contextlib import ExitStack

import concourse.bass as bass
import concourse.mybir as mybir
import concourse.tile as tile
from concourse.tile import add_dep_helper
from concourse.bass_utils import run_bass_kernel_spmd

N_CORES = 8
B, C, M, N = 32, 256, 56, 56
L, K = 18, 18
KS = 3
ALPHA = 0.1

P = 128                      # SBUF partitions
ROWS = (B // N_CORES) * C    # images per core = 1024
PIX = M * N                  # 3136
PATCH = L * K                # 324
NT = ROWS // P               # 8 tiles per core

F32 = mybir.dt.float32
I32 = mybir.dt.int32
I8 = mybir.dt.int8
AX = mybir.AxisListType.X
MAX = mybir.AluOpType.max


def build_nc():
    nc = bass.Bass("TRN2", target_bir_lowering=False, debug=False)

    act_d = nc.dram_tensor("act", [ROWS, PIX], F32, kind="ExternalInput")
    rc_d = nc.dram_tensor("rc", [ROWS, PIX], I32, kind="ExternalInput")
    patch_d = nc.dram_tensor("patch", [ROWS, PATCH], I32, kind="ExternalInput")
    out_d = nc.dram_tensor("out", [ROWS, PIX], F32, kind="ExternalOutput")

    with tile.TileContext(nc) as tc, ExitStack() as ctx:
        actp = ctx.enter_context(tc.tile_pool(name="actp", bufs=NT))
        rcp = ctx.enter_context(tc.tile_pool(name="rcp", bufs=2))
        pp = ctx.enter_context(tc.tile_pool(name="pp", bufs=2))
        mp = ctx.enter_context(tc.tile_pool(name="mp", bufs=2))
        tp = ctx.enter_context(tc.tile_pool(name="tp", bufs=2))
        # tiny tiles fully buffered: reuse would add WAW self-waits
        sp = ctx.enter_context(tc.tile_pool(name="sp", bufs=NT))
        scrp = ctx.enter_context(tc.tile_pool(name="scrp", bufs=NT))

        obs_tail = [None] * NT    # last Pool observer per iteration
        scr_d_hist = [None] * NT  # Pool-written observer tiles
        tenth_hist = [None] * NT
        act_hist = [None] * NT
        jfin_hist = [None] * NT
        store_hist = [None] * NT
        dma_hist = []
        final_insts = []
        pend_obs = None           # (tenth, act, t) awaiting deferred emission

        def emit_deferred_obs(after_inst):
            """Pool observers of iteration t's ACT/DVE ticks plus its store,
            emitted after iteration t+1's loads so the DMA queue never
            stalls on live compute."""
            nonlocal pend_obs
            if pend_obs is None:
                return
            tenth_o, act_o, to = pend_obs
            scr_t = scrp.tile([P, 1], F32, tag="scr_t")
            o4 = nc.gpsimd.tensor_copy(scr_t[:, :], tenth_o[:, 0:1])
            add_dep_helper(o4.ins, after_inst, sync=False, reason="obs defer")
            scr_d = scrp.tile([P, 1], F32, tag="scr_d")
            o5 = nc.gpsimd.tensor_copy(scr_d[:, :], act_o[:, 0:1])
            add_dep_helper(o5.ins, o4.ins, sync=False, reason="obs chain")
            i_st = nc.gpsimd.dma_start(
                out=out_d.ap()[to * P : to * P + P, :], in_=act_o[:, :]
            )
            add_dep_helper(i_st.ins, o5.ins, sync=False, reason="store after obs")
            store_hist[to] = i_st
            obs_tail[to] = o5.ins
            scr_d_hist[to] = scr_d
            pend_obs = None
            if to == NT - 1:
                final_insts.append(o5)
                final_insts.append(i_st)

        for t in range(NT):
            r0 = t * P

            patch = pp.tile([P, PATCH], I32, tag="patch")
            i_lp = nc.gpsimd.dma_start(
                out=patch[:, :], in_=patch_d.ap()[r0 : r0 + P, :]
            )
            act = actp.tile([P, PIX], F32, tag="act")
            i_la = nc.gpsimd.dma_start(out=act[:, :], in_=act_d.ap()[r0 : r0 + P, :])
            add_dep_helper(i_la.ins, i_lp.ins, sync=False, reason="load order")
            rc = rcp.tile([P, PIX], I32, tag="rc")
            i_lr = nc.gpsimd.dma_start(out=rc[:, :], in_=rc_d.ap()[r0 : r0 + P, :])
            add_dep_helper(i_lr.ins, i_la.ins, sync=False, reason="load order")
            if t >= 2:
                for ld in (i_la, i_lr, i_lp):
                    add_dep_helper(
                        ld.ins, obs_tail[t - 2], sync=False, reason="load after obs"
                    )

            # Pool observers of the three load lanes (in arrival order)
            scr_p = scrp.tile([P, 1], I32, tag="scr_p")
            op_ = nc.gpsimd.tensor_copy(scr_p[:, :], patch[:, 0:1])
            add_dep_helper(op_.ins, i_lr.ins, sync=False, reason="obs after loads")
            scr_a = scrp.tile([P, 1], F32, tag="scr_a")
            o1 = nc.gpsimd.tensor_copy(scr_a[:, :], act[:, 0:1])
            add_dep_helper(o1.ins, op_.ins, sync=False, reason="obs chain")
            scr_r = scrp.tile([P, 1], I32, tag="scr_r")
            o3 = nc.gpsimd.tensor_copy(scr_r[:, :], rc[:, 0:1])
            add_dep_helper(o3.ins, o1.ins, sync=False, reason="obs chain")

            # previous iteration's compute observers go here, AFTER this
            # iteration's loads have entered the queue
            emit_deferred_obs(o3.ins)

            # per-image spatial peak
            peak = sp.tile([P, 1], F32, tag="peak")
            nc.vector.tensor_reduce(peak[:, :], act[:, :], axis=AX, op=MAX)

            # peak indicator (int8: CopyPredicated needs an integer mask)
            pcm = mp.tile([P, PIX], I8, tag="pcm")
            nc.vector.tensor_scalar(
                pcm[:, :], act[:, :], peak[:, :], None, mybir.AluOpType.is_ge
            )

            # ACT joiners: observe Pool (t-2), own sem (t-1), DVE (t-2); all
            # two-iterations-old targets, so ACT never waits on live compute
            chain = []
            if t >= 1:
                scr_ap = scrp.tile([P, 1], F32, tag="scr_ap")
                chain.append(nc.scalar.copy(scr_ap[:, :], scr_d_hist[t - 1][:, 0:1]))
                scr_as = scrp.tile([P, 1], F32, tag="scr_as")
                chain.append(nc.scalar.copy(scr_as[:, :], tenth_hist[t - 1][:, 0:1]))
            scr_al = scrp.tile([P, 1], F32, tag="scr_al")
            chain.append(nc.scalar.copy(scr_al[:, :], act[:, 0:1]))
            if t >= 1:
                scr_s = scrp.tile([P, 1], F32, tag="scr_s")
                chain.append(nc.scalar.copy(scr_s[:, :], jfin_hist[t - 1][:, 0:1]))
            for x, y in zip(chain[1:], chain[:-1]):
                add_dep_helper(x.ins, y.ins, sync=False, reason="act chain")
            prev_a = chain[-1]

            # expand patch mask 18x18 -> 56x56 in three scalar-engine ops
            # (one per repeated row): the input AP repeats each patch element
            # 3x along columns via a zero-step dim (ISA allows 3 free dims)
            mask = mp.tile([P, PIX], I8, tag="mask")
            m3 = mask[:, :].rearrange("p (r c) -> p r c", r=M)
            pap = patch[:, :]
            axp = prev_a
            for i in range(KS):
                in4 = bass.AP(
                    tensor=pap.tensor,
                    offset=pap.offset,
                    ap=[list(pap.ap[0]), [K, L], [1, K], [0, KS]],
                )
                out4 = bass.AP(
                    tensor=m3.tensor,
                    offset=m3.offset + i * N,
                    ap=[list(m3.ap[0]), [N * KS, L], [KS, K], [1, KS]],
                )
                nx = nc.scalar.copy(out4, in4)
                if axp is not None:
                    add_dep_helper(nx.ins, axp.ins, sync=False, reason="act chain")
                axp = nx
            # border zeros via x*0 (junk int8 input is finite, so exact)
            br = mask[:, L * KS * N :]
            az1 = nc.scalar.mul(br, br, 0.0)                 # bottom rows
            add_dep_helper(az1.ins, axp.ins, sync=False, reason="act chain")
            bc = m3[:, 0 : L * KS, K * KS : N]
            az2 = nc.scalar.mul(bc, bc, 0.0)                 # right cols
            add_dep_helper(az2.ins, az1.ins, sync=False, reason="act chain")

            # DVE observer of Pool's o1 read of act, then a 1-element
            # self-copy that soaks up the load-lane WAW, so the in-place cp2
            # below needs only its own-engine wait (placed here to fill the
            # wait-for-mask window)
            j_o1 = sp.tile([P, 1], F32, tag="j_o1")
            jo = nc.vector.tensor_reduce(j_o1[:, :], scr_a[:, :], axis=AX, op=MAX)
            j_a2 = sp.tile([P, 1], F32, tag="j_a2")
            ja2 = nc.vector.tensor_reduce(j_a2[:, :], scr_al[:, :], axis=AX, op=MAX)
            add_dep_helper(ja2.ins, jo.ins, sync=False, reason="j chain")
            j_w = nc.vector.tensor_copy(act[:, 0:1], act[:, 0:1])
            add_dep_helper(j_w.ins, ja2.ins, sync=False, reason="selfcopy after j")

            # DVE joiners ahead of the copy_predicateds
            j_mask = sp.tile([P, 1], I8, tag="j_mask")
            nc.vector.tensor_reduce(j_mask[:, :], mask[:, PIX - 116 :], axis=AX, op=MAX)
            j_rc = sp.tile([P, 1], I32, tag="j_rc")
            nc.vector.tensor_reduce(j_rc[:, :], rc[:, 0:8], axis=AX, op=MAX)

            # at peak pixels the mask comes from rc instead
            nc.vector.copy_predicated(mask[:, :], pcm[:, :], rc[:, :])

            tenth = tp.tile([P, PIX], F32, tag="tenth")
            at = nc.scalar.mul(tenth[:, :], act[:, :], ALPHA)
            add_dep_helper(at.ins, az2.ins, sync=False, reason="act chain")
            j_tenth = sp.tile([P, 1], F32, tag="j_tenth")
            nc.vector.tensor_reduce(j_tenth[:, :], tenth[:, 0:8], axis=AX, op=MAX)

            cp2 = nc.vector.copy_predicated(act[:, :], mask[:, :], tenth[:, :])
            add_dep_helper(cp2.ins, j_w.ins, sync=False, reason="cp2 after j_w")
            # tiny DVE tail marker: gives later iterations an observable
            # DVE-written tile whose tick is past cp2
            j_fin = sp.tile([P, 1], F32, tag="j_fin")
            jf = nc.vector.tensor_reduce(j_fin[:, :], act[:, 0:4], axis=AX, op=MAX)
            jfin_hist[t] = j_fin

            tenth_hist[t] = tenth
            act_hist[t] = act
            pend_obs = (tenth, act, t)
            dma_hist.append((t, i_la))
            dma_hist.append((t, i_lr))
            dma_hist.append((t, i_lp))
            if t == NT - 1:
                final_insts = [at, jf]

        # flush the last iteration's observers + store
        emit_deferred_obs(obs_tail[NT - 2])

        # Feed SP's vector clock the final value of every live semaphore so
        # the kernel-tail drain needs no multi-wait.
        targets = (
            [ins for (tt, ins) in dma_hist if tt >= NT - 3]
            + [s for s in store_hist if s is not None][-3:]
            + final_insts
        )
        prev = None
        for tgt in targets:
            nop = nc.sync.nop()
            add_dep_helper(nop.ins, tgt.ins, sync=True, reason="drain prefetch")
            if prev is not None:
                add_dep_helper(nop.ins, prev.ins, sync=False, reason="nop chain")
            prev = nop

    return nc


_NC_CACHE = None


def _get_nc():
    global _NC_CACHE
    if _NC_CACHE is None:
        _NC_CACHE = build_nc()
    return _NC_CACHE


def shard_inputs(activation, rc, p_patch_mask):
    bs = B // N_CORES
    in_maps = []
    for i in range(N_CORES):
        sl = slice(i * bs, (i + 1) * bs)
        in_maps.append(
            {
                "act": np.ascontiguousarray(
                    activation[sl].reshape(ROWS, PIX), dtype=np.float32
                ),
                "rc": np.ascontiguousarray(rc[sl].reshape(ROWS, PIX), dtype=np.int32),
                "patch": np.ascontiguousarray(
                    p_patch_mask[sl].reshape(ROWS, PATCH), dtype=np.int32
                ),
            }
        )
    return in_maps


def kernel(activation, rc, p_patch_mask, _trace=False, _trace_kwargs=None):
    activation = np.asarray(activation)
    rc = np.asarray(rc)
    p_patch_mask = np.asarray(p_patch_mask)

    nc = _get_nc()
    in_maps = shard_inputs(activation, rc, p_patch_mask)
    res = run_bass_kernel_spmd(
        nc,
        in_maps,
        core_ids=list(range(N_CORES)),
        trace=_trace,
        **(_trace_kwargs or {}),
    )
    bs = B // N_CORES
    out = np.concatenate(
        [res.results[i]["out"].reshape(bs, C, M, N) for i in range(N_CORES)], axis=0
    )
    if _trace:
        return out, res
    return out

